# revision 1
# baseline (speedup 1.0000x reference)
"""Causal multi-head attention block on 8 Trainium2 NeuronCores.

Reference computation (per batch b):
    qkv = x @ w_attn + b_attn; split q,k,v; per head: S = q k^T / sqrt(hs),
    causal softmax, y = P v; out = concat(y) @ w_proj + b_proj.

Sharding: data parallel over batch. B == 8 == n_cores, so each core runs one
batch independently (no collectives). Each core gets the full weights and its
x[b] slice; outputs are stacked on the host.

Per-core dataflow (all matmuls in float32r: full PE rate, ~1e-4 rounding):
  xT = transpose(x) via PE                              [C, T]
  v   = lhsT=xT tiles, rhs=w_attn[:, 2C:] -> natural [T, C] (+ ones column
        for the softmax denominator trick)
  qkT = lhsT=w_attn[:, :2C] tiles, rhs=xT -> [2C, T]
        (n on partitions => per-head qT_h/kT_h are [64, T] slices)
  attention per head h over chunk-pairs cp in {(0,1), (2,3)} (i-chunks of 256):
      for j-tile pair a, chunk c (only causal-reachable blocks):
        ST pair tile [128, 512] <- two QK matmuls (jt=2a | jt=2a+1)
        PT = exp(0.125 * ST) in one ACT op; 0/1-mask the diagonal tiles
        ya[c] [65, 256] += v_aug_h[jt]^T @ PT halves   (row 64 = row sums l)
      rl = approx 1/l; gpsimd-broadcast; yT = ya * rl
  out = lhsT=yT tiles, rhs=w_proj + b_proj -> [T, C]
"""

import numpy as np
from contextlib import ExitStack

import concourse.bass as bass
import concourse.mybir as mybir
import concourse.tile as tile
from concourse import bacc
from concourse.bass_utils import run_bass_kernel_spmd
from concourse.masks import make_identity

F32 = mybir.dt.float32
F32R = mybir.dt.float32r
AF = mybir.ActivationFunctionType

B, T, C = 8, 1024, 768
H, HS = 12, 64
KT = C // 128            # 6 contraction tiles
MT = T // 128            # 8 row tiles (also j tiles)
ICH = 256                # attention i-chunk width
NCH = T // ICH           # 4 chunks
SCALE = 1.0 / np.sqrt(HS)

N_CORES = 8


def build_program():
    nc = bacc.Bacc("TRN2", target_bir_lowering=False, debug=False)

    x = nc.dram_tensor("x", [T, C], F32, kind="ExternalInput")
    w_attn = nc.dram_tensor("w_attn", [C, 3 * C], F32, kind="ExternalInput")
    b_attn = nc.dram_tensor("b_attn", [3 * C], F32, kind="ExternalInput")
    w_proj = nc.dram_tensor("w_proj", [C, C], F32, kind="ExternalInput")
    b_proj = nc.dram_tensor("b_proj", [C], F32, kind="ExternalInput")
    out = nc.dram_tensor("out", [T, C], F32, kind="ExternalOutput")

    with tile.TileContext(nc) as tc, ExitStack() as ctx:
        consts = ctx.enter_context(tc.tile_pool(name="consts", bufs=1))
        big = ctx.enter_context(tc.tile_pool(name="big", bufs=1))
        io = ctx.enter_context(tc.tile_pool(name="io", bufs=2))
        wstage = ctx.enter_context(tc.tile_pool(name="wstage", bufs=2))
        pt_pool = ctx.enter_context(tc.tile_pool(name="pt", bufs=6))
        rl_pool = ctx.enter_context(tc.tile_pool(name="rl", bufs=3))
        ps_st = ctx.enter_context(tc.tile_pool(name="ps_st", bufs=3, space="PSUM"))
        ps_big = ctx.enter_context(tc.tile_pool(name="ps_big", bufs=2, space="PSUM"))
        ps_y = ctx.enter_context(tc.tile_pool(name="ps_y", bufs=3, space="PSUM"))

        # ---- constants ----
        ident = consts.tile([128, 128], F32, tag="ident")
        make_identity(nc, ident)
        # tri: 1.0 where f >= p (keep j <= i on an exact-diagonal block)
        tri = consts.tile([128, 128], F32, tag="tri")
        nc.gpsimd.memset(tri, 1.0)
        nc.gpsimd.affine_select(
            out=tri, in_=tri, compare_op=mybir.AluOpType.is_ge,
            fill=0.0, base=0, pattern=[[1, 128]], channel_multiplier=-1,
        )
        # mask1 [128, 256]: zeros | tri  (for the jt=2a+1 half of a diagonal
        # pair tile: first 128 i-cols fully masked, next 128 triangular)
        mask1 = consts.tile([128, 256], F32, tag="mask1")
        nc.gpsimd.memset(mask1[:, 0:128], 0.0)
        nc.gpsimd.memset(mask1[:, 128:256], 1.0)
        nc.gpsimd.affine_select(
            out=mask1[:, 128:256], in_=mask1[:, 128:256],
            compare_op=mybir.AluOpType.is_ge,
            fill=0.0, base=0, pattern=[[1, 128]], channel_multiplier=-1,
        )
        battn_t = consts.tile([128, 18], F32, tag="battn_t")
        nc.sync.dma_start(out=battn_t, in_=b_attn[:].rearrange("(t p) -> p t", p=128))
        bv_b = consts.tile([128, C], F32, tag="bias_b")
        nc.sync.dma_start(
            out=bv_b,
            in_=bass.AP(tensor=b_attn[:].tensor, offset=2 * C, ap=[[0, 128], [1, C]]),
        )
        ones12 = consts.tile([128, H], F32, tag="ones12")
        nc.gpsimd.memset(ones12, 1.0)

        # ---- phase A: x load + transpose (streamed per m-tile) ----
        xT = big.tile([128, KT, T], F32R, tag="xT")
        for mt in range(MT):
            x_sb = io.tile([128, C], F32, tag="io")
            nc.sync.dma_start(out=x_sb, in_=x[mt * 128:(mt + 1) * 128, :])
            for kt in range(KT):
                pt = ps_st.tile([128, 512], F32, tag="st")
                nc.tensor.transpose(
                    pt[:, 0:128], x_sb[:, kt * 128:(kt + 1) * 128], ident)
                dst = xT[:, kt, mt * 128:(mt + 1) * 128]
                if (mt * KT + kt) % 2 == 0:
                    nc.vector.tensor_copy(dst, pt[:, 0:128])
                else:
                    nc.scalar.copy(dst, pt[:, 0:128])

        # ---- weights: DMA chunk into f32 staging, round to f32r (DVE/ACT) ----
        WCH = 1152
        wr = big.tile([128, KT, 3 * C], F32R, tag="w_sb")
        w_dram = w_attn[:].rearrange("(t p) n -> p t n", p=128)
        cast_i = 0
        for kt in range(KT):
            for c0 in range(0, 3 * C, WCH):
                stg = wstage.tile([128, WCH], F32, tag="wstage")
                nc.sync.dma_start(out=stg, in_=w_dram[:, kt, c0:c0 + WCH])
                if cast_i % 2 == 0:
                    nc.vector.tensor_copy(wr[:, kt, c0:c0 + WCH], stg)
                else:
                    nc.scalar.copy(wr[:, kt, c0:c0 + WCH], stg)
                cast_i += 1

        wpr = big.tile([128, KT, C], F32R, tag="wp_sb")
        wp_dram = w_proj[:].rearrange("(t p) n -> p t n", p=128)
        for kt in range(KT):
            stg = wstage.tile([128, WCH], F32, tag="wstage")
            nc.sync.dma_start(out=stg[:, 0:C], in_=wp_dram[:, kt, :])
            if cast_i % 2 == 0:
                nc.vector.tensor_copy(wpr[:, kt, :], stg[:, 0:C])
            else:
                nc.scalar.copy(wpr[:, kt, :], stg[:, 0:C])
            cast_i += 1

        # ---- phase B1: v_aug [T, H, 65] (natural layout + ones column) ----
        v_aug = big.tile([128, MT, H, HS + 1], F32R, tag="v_aug")
        for mt in range(MT):
            nc.vector.tensor_copy(
                v_aug[:, mt, :, HS:HS + 1].rearrange("p h o -> p (h o)"), ones12)
            for (n0, nsz) in [(0, 512), (512, 256)]:
                ps = ps_big.tile([128, 512], F32, tag="mm")
                for kt in range(KT):
                    nc.tensor.matmul(
                        ps[:, 0:nsz], xT[:, kt, mt * 128:(mt + 1) * 128],
                        wr[:, kt, 2 * C + n0:2 * C + n0 + nsz],
                        start=(kt == 0), stop=(kt == KT - 1),
                    )
                h0, nh = n0 // HS, nsz // HS
                nc.vector.tensor_add(
                    v_aug[:, mt, h0:h0 + nh, 0:HS],
                    ps[:, 0:nsz].rearrange("p (h d) -> p h d", d=HS),
                    bv_b[:, n0:n0 + nsz].rearrange("p (h d) -> p h d", d=HS),
                )

        # ---- phase B2: qkT [2C, T], in head-pair order so heads start early --
        qk = big.tile([128, 12, T], F32R, tag="qk")
        for pair in range(6):
            for nt in (pair, 6 + pair):      # q tile then k tile for this pair
                for mc in range(2):
                    ps = ps_big.tile([128, 512], F32, tag="mm")
                    for kt in range(KT):
                        nc.tensor.matmul(
                            ps, wr[:, kt, nt * 128:(nt + 1) * 128],
                            xT[:, kt, mc * 512:(mc + 1) * 512],
                            start=(kt == 0), stop=(kt == KT - 1),
                        )
                    nc.scalar.activation(
                        qk[:, nt, mc * 512:(mc + 1) * 512], ps, AF.Identity,
                        bias=battn_t[:, nt:nt + 1], scale=1.0,
                    )

        # ---- phase C: attention per head over chunk-pairs ----
        yT = big.tile([128, KT, T], F32R, tag="xT")  # reuses xT's slot
        for h in range(H):
            nt_q, po = h // 2, 64 * (h % 2)
            qT_h = qk[po:po + 64, nt_q, :]
            kT_h = qk[po:po + 64, 6 + nt_q, :]
            for cp in ((0, 1), (2, 3)):
                ya_pair = ps_y.tile([HS + 1, 2 * ICH], F32, tag="ya",
                                    name=f"ya_{h}_{cp[0]}")
                yas = {c: ya_pair[:, (c - cp[0]) * ICH:(c - cp[0] + 1) * ICH]
                       for c in cp}
                # (a, c) block-pair units, c-major: chunk c's psum accumulation
                # group must close before c+1's opens (shared ya bank)
                units = [(a, c) for c in cp for a in range(c + 1)]
                for a, c in units:
                    isl = slice(c * ICH, (c + 1) * ICH)
                    st = ps_st.tile([128, 512], F32, tag="st")
                    for half, jt in enumerate((2 * a, 2 * a + 1)):
                        nc.tensor.matmul(
                            st[:, half * 256:(half + 1) * 256],
                            kT_h[:, jt * 128:(jt + 1) * 128], qT_h[:, isl],
                            start=True, stop=True,
                        )
                    ptile = pt_pool.tile([128, 512], F32R, tag="ptile")
                    nc.scalar.activation(ptile, st, AF.Exp, bias=0.0, scale=SCALE)
                    if a == c:  # diagonal pair tile
                        nc.vector.tensor_mul(ptile[:, 0:128], ptile[:, 0:128], tri)
                        nc.vector.tensor_mul(
                            ptile[:, 256:512], ptile[:, 256:512], mask1)
                    for half, jt in enumerate((2 * a, 2 * a + 1)):
                        nc.tensor.matmul(
                            yas[c], v_aug[:, jt, h, :],
                            ptile[:, half * 256:(half + 1) * 256],
                            start=(a == 0 and half == 0),
                            stop=(a == c and half == 1),
                        )
                for c in cp:
                    isl = slice(c * ICH, (c + 1) * ICH)
                    ya = yas[c]
                    rl = rl_pool.tile([1, ICH], F32, tag="rl")
                    nc.vector.reciprocal(rl, ya[HS:HS + 1, :])
                    rlb = rl_pool.tile([64, ICH], F32, tag="rlb")
                    nc.gpsimd.partition_broadcast(rlb, rl)
                    nc.vector.tensor_mul(
                        yT[po:po + 64, nt_q, isl], ya[0:HS, :], rlb)

        # ---- phase D: output projection (streamed per m-tile) ----
        bp_b = consts.tile([128, C], F32, tag="bias_b")  # reuses bv_b's slot
        nc.sync.dma_start(
            out=bp_b,
            in_=bass.AP(tensor=b_proj[:].tensor, offset=0, ap=[[0, 128], [1, C]]),
        )
        for mt in range(MT):
            out_sb = io.tile([128, C], F32, tag="io")
            for (c0, csz) in [(0, 512), (512, 256)]:
                ps = ps_big.tile([128, 512], F32, tag="mm")
                for nt in range(KT):
                    nc.tensor.matmul(
                        ps[:, 0:csz], yT[:, nt, mt * 128:(mt + 1) * 128],
                        wpr[:, nt, c0:c0 + csz],
                        start=(nt == 0), stop=(nt == KT - 1),
                    )
                nc.vector.tensor_add(
                    out_sb[:, c0:c0 + csz], ps[:, 0:csz], bp_b[:, c0:c0 + csz])
            nc.sync.dma_start(
                out=out[mt * 128:(mt + 1) * 128, :], in_=out_sb)

    nc.compile()
    return nc


_CACHE = {}


def _get_program():
    if "nc" not in _CACHE:
        _CACHE["nc"] = build_program()
    return _CACHE["nc"]


def kernel(x, w_attn, b_attn, w_proj, b_proj):
    nc = _get_program()
    x = np.asarray(x, dtype=np.float32)
    in_maps = [
        {
            "x": np.ascontiguousarray(x[b]),
            "w_attn": np.asarray(w_attn, np.float32),
            "b_attn": np.asarray(b_attn, np.float32),
            "w_proj": np.asarray(w_proj, np.float32),
            "b_proj": np.asarray(b_proj, np.float32),
        }
        for b in range(B)
    ]
    res = run_bass_kernel_spmd(nc, in_maps, list(range(N_CORES)))
    return np.stack([res.results[b]["out"] for b in range(B)], axis=0)



# revision 39
# speedup vs baseline: 1.6167x; 1.6167x over previous
"""Causal multi-head attention block on 8 Trainium2 NeuronCores.

Reference computation (per batch b):
    qkv = x @ w_attn + b_attn; split q,k,v; per head: S = q k^T / sqrt(hs),
    causal softmax, y = P v; out = concat(y) @ w_proj + b_proj.

Sharding: data parallel over batch. B == 8 == n_cores, so each core runs one
batch independently (no collectives).

Per-core dataflow (matmul operands bf16, accumulation f32, ~3.5e-3 rel err):
  A:  x DMA'd per m-tile on the sync queue (x0,x1 -> w_v -> x2..7 -> w_qk ->
      w_proj ordering keeps consumers fed); PE-transposes build xT bf16.
      Weights stage through SBUF f32 then round to bf16 on DVE/gpsimd/ACT.
  B1: v = x @ w_attn[:,2C:] + b -> v_aug bf16 [T,H,65] (ones column yields
      the softmax denominator l as PV output row 64).
  B2: qkT = w_attn[:,:2C]^T x^T -> qk bf16 [2C,T].  Head pair 2p,2p+1 sits at
      partitions 0:64/64:128, so their contraction-64 QK matmuls auto-derive
      tile_position (0,0)/(64,0) and run CONCURRENTLY in PE row groups
      (outputs must land in different PSUM banks - same-bank concurrent PE
      writes hang the device).
  C:  global software-pipelined stream over (pair, chunk c, j-tile-pair a)
      double-units, lookahead 3 (PV of du i issues after QK of du i+3, so the
      PE never head-of-line blocks on the exp).  Each du: 4 QK matmuls ->
      ST [128,1024] (head h in bank 0, h+1 in bank 1) -> one [128,1024] Exp
      ACTIVATE -> ptile bf16 -> causal masks (DVE) on diagonal dus -> 4 PV
      matmuls accumulate both heads into ONE ya bank [65,512] (single
      has_written clear on the chunk's first matmul, accumulate-only after).
      Chunk tail: l row -> SBUF (custom-DVE recip is SBUF-only on HW) ->
      reciprocal_approx_fast -> gpsimd partition-broadcast -> DVE muls write
      normalized yT bf16.  The next pair's B2 groups fill PE gaps at chunk
      closes.
  D:  out = yT^T @ w_proj + b_proj per m-tile, streamed out on sync queue.
"""

import numpy as np
from contextlib import ExitStack

import concourse.bass as bass
import concourse.mybir as mybir
import concourse.tile as tile
from concourse import bacc
from concourse.bass_utils import run_bass_kernel_spmd
from concourse.masks import make_identity

F32 = mybir.dt.float32
F32R = mybir.dt.float32r
BF16 = mybir.dt.bfloat16
AF = mybir.ActivationFunctionType

B, T, C = 8, 1024, 768
H, HS = 12, 64
KT = C // 128            # 6 contraction tiles
MT = T // 128            # 8 row tiles (also j tiles)
ICH = 256                # attention i-chunk width
NCH = T // ICH           # 4 chunks
SCALE = 1.0 / np.sqrt(HS)

N_CORES = 8


def build_program():
    nc = bacc.Bacc("TRN2", target_bir_lowering=False, debug=False)

    x = nc.dram_tensor("x", [T, C], F32, kind="ExternalInput")
    w_attn = nc.dram_tensor("w_attn", [C, 3 * C], F32, kind="ExternalInput")
    b_attn = nc.dram_tensor("b_attn", [3 * C], F32, kind="ExternalInput")
    w_proj = nc.dram_tensor("w_proj", [C, C], F32, kind="ExternalInput")
    b_proj = nc.dram_tensor("b_proj", [C], F32, kind="ExternalInput")
    out = nc.dram_tensor("out", [T, C], F32, kind="ExternalOutput")

    with tile.TileContext(nc) as tc, ExitStack() as ctx:
        consts = ctx.enter_context(tc.tile_pool(name="consts", bufs=1))
        big = ctx.enter_context(tc.tile_pool(name="big", bufs=1))
        io = ctx.enter_context(tc.tile_pool(name="io", bufs=2))
        wstage = ctx.enter_context(tc.tile_pool(name="wstage", bufs=2))
        pt_pool = ctx.enter_context(tc.tile_pool(name="pt", bufs=4))
        rl_pool = ctx.enter_context(tc.tile_pool(name="rl", bufs=3))
        # PSUM: st 2x[128,1024] = 4 banks, ya 3x[65,256] = 3, mm 1x[128,512] = 1
        ps_st = ctx.enter_context(tc.tile_pool(name="ps_st", bufs=2, space="PSUM"))
        ps_ya = ctx.enter_context(tc.tile_pool(name="ps_ya", bufs=3, space="PSUM"))
        ps_mm = ctx.enter_context(tc.tile_pool(name="ps_mm", bufs=1, space="PSUM"))

        # ---- constants ----
        ident = consts.tile([128, 128], F32, tag="ident")
        make_identity(nc, ident)
        # One head's diagonal du occupies 512 ptile cols laid out
        # [jt=2c: i0|i1, jt=2c+1: i0|i1] -> causal mask [tri|ones|zero|tri].
        maskC2 = consts.tile([128, 1024], BF16, tag="maskC2")
        nc.gpsimd.memset(maskC2, 1.0)
        for h0 in (0, 512):
            nc.gpsimd.memset(maskC2[:, h0 + 256:h0 + 384], 0.0)
            for c0 in (h0, h0 + 384):
                nc.gpsimd.affine_select(
                    out=maskC2[:, c0:c0 + 128], in_=maskC2[:, c0:c0 + 128],
                    compare_op=mybir.AluOpType.is_ge,
                    fill=0.0, base=0, pattern=[[1, 128]],
                    channel_multiplier=-1,
                )
        # weight/bias stream on the ACT hwdge queue, x/out on the sync queue
        battn_t = consts.tile([128, 18], F32, tag="battn_t")
        nc.sync.dma_start(out=battn_t, in_=b_attn[:].rearrange("(t p) -> p t", p=128))
        bv_b = consts.tile([128, C], F32, tag="bv_b")
        nc.sync.dma_start(
            out=bv_b,
            in_=bass.AP(tensor=b_attn[:].tensor, offset=2 * C, ap=[[0, 128], [1, C]]),
        )
        bp_b = consts.tile([128, C], F32, tag="bp_b")
        nc.sync.dma_start(
            out=bp_b,
            in_=bass.AP(tensor=b_proj[:].tensor, offset=0, ap=[[0, 128], [1, C]]),
        )

        # ---- big SBUF tensors (bf16 for all matmul operands) ----
        xT = big.tile([128, KT, T], BF16, tag="xT")
        wr = big.tile([128, KT, 3 * C], BF16, tag="wr")
        wpr = big.tile([128, KT, C], BF16, tag="wpr")
        qk = big.tile([128, 12, T], BF16, tag="qk")
        v_aug = big.tile([128, MT, H, HS + 1], BF16, tag="v_aug")
        yT = big.tile([128, KT, T], BF16, tag="yT")

        w_dram = w_attn[:].rearrange("(t p) n -> p t n", p=128)
        wp_dram = w_proj[:].rearrange("(t p) n -> p t n", p=128)

        # Weights: DMA f32 into staging (per-class tags, enough bufs that the
        # DMA stream never blocks on cast completion), round f32->bf16 in
        # pieces spread over DVE / gpsimd / ACT.
        wcast_i = 0

        def stage_w(dst, src, n, tag, bufs, size):
            nonlocal wcast_i
            stg = wstage.tile([128, size], F32, tag=tag, bufs=bufs,
                              name=f"wstg_{n}")
            sview = stg[:, 0:src.free_size()]
            if len(src.shape) == 3:
                sview = sview.rearrange("p (t n) -> p t n", n=src.shape[2])
            nc.sync.dma_start(out=sview, in_=src)
            pieces = src.shape[1] if len(src.shape) == 3 else 2
            for i in range(pieces):
                d = dst[:, i] if len(src.shape) == 3 else \
                    dst[:, i * (dst.shape[1] // 2):(i + 1) * (dst.shape[1] // 2)]
                s = sview[:, i] if len(src.shape) == 3 else \
                    sview[:, i * (sview.shape[1] // 2):(i + 1) * (sview.shape[1] // 2)]
                eng = (nc.vector, nc.gpsimd, nc.scalar)[wcast_i % 3]
                if eng is nc.scalar:
                    nc.scalar.copy(d, s)
                else:
                    eng.tensor_copy(d, s)
                wcast_i += 1

        # ---- phase A + weight streaming, ordered for the single DMA queue:
        # x0,x1 -> v cols -> x2..7 -> q/k halves -> w_proj
        def a_tile(mt):
            x_sb = io.tile([128, C], F32, tag="io", name=f"x_sb_{mt}")
            nc.sync.dma_start(out=x_sb, in_=x[mt * 128:(mt + 1) * 128, :])
            pt = ps_st.tile([128, 1024], F32, tag="st", name=f"xt_ps_{mt}")
            for kt in range(KT):
                nc.tensor.transpose(
                    pt[:, kt * 128:(kt + 1) * 128],
                    x_sb[:, kt * 128:(kt + 1) * 128], ident)
            if mt % 2 == 0:
                nc.vector.tensor_copy(
                    xT[:, :, mt * 128:(mt + 1) * 128],
                    pt[:, 0:C].rearrange("p (k c) -> p k c", c=128))
            else:
                nc.scalar.copy(
                    xT[:, :, mt * 128:(mt + 1) * 128],
                    pt[:, 0:C].rearrange("p (k c) -> p k c", c=128))

        a_tile(0)
        a_tile(1)
        for kt in range(KT):
            stage_w(wr[:, kt, 2 * C:3 * C], w_dram[:, kt, 2 * C:3 * C],
                    f"v{kt}", "wv", 4, 768)
        for mt in range(2, MT):
            a_tile(mt)
        for half in range(2):
            for n0 in (0, C):                # q half then k half
                sl = slice(n0 + half * 384, n0 + (half + 1) * 384)
                stage_w(wr[:, :, sl], w_dram[:, :, sl], f"qk{half}_{n0}",
                        "wqk", 2, 2304)
        for kt in range(KT):
            stage_w(wpr[:, kt, :], wp_dram[:, kt, :], f"wp{kt}", "wv", 4, 768)

        # ---- phase B1/B2/D emitters ----
        def emit_b1(mt, ps2):
            # ps2: two APs [128,512] in distinct banks (groups for n0=0/512)
            nc.gpsimd.memset(
                v_aug[:, mt, :, HS:HS + 1].rearrange("p h o -> p (h o)"), 1.0)
            for gi, (n0, nsz) in enumerate([(0, 512), (512, 256)]):
                for kt in range(KT):
                    nc.tensor.matmul(
                        ps2[gi][:, 0:nsz],
                        xT[:, kt, mt * 128:(mt + 1) * 128],
                        wr[:, kt, 2 * C + n0:2 * C + n0 + nsz],
                        start=(kt == 0), stop=(kt == KT - 1),
                    )
            for gi, (n0, nsz) in enumerate([(0, 512), (512, 256)]):
                h0, nh = n0 // HS, nsz // HS
                nc.vector.tensor_add(
                    v_aug[:, mt, h0:h0 + nh, 0:HS],
                    ps2[gi][:, 0:nsz].rearrange("p (h d) -> p h d", d=HS),
                    bv_b[:, n0:n0 + nsz].rearrange("p (h d) -> p h d", d=HS),
                )

        def b1_st(mt):
            ps = ps_st.tile([128, 1024], F32, tag="st", name=f"v_ps_{mt}")
            emit_b1(mt, [ps[:, 0:512], ps[:, 512:1024]])

        def b1_mm(mt):
            def go():
                ps2 = [ps_mm.tile([128, 512], F32, tag="mm",
                                  name=f"v_mm_{mt}_{g}") for g in range(2)]
                emit_b1(mt, ps2)
            return go

        def emit_b2_group(nt, mc, ps, on_act=False):
            for kt in range(KT):
                nc.tensor.matmul(
                    ps, wr[:, kt, nt * 128:(nt + 1) * 128],
                    xT[:, kt, mc * 512:(mc + 1) * 512],
                    start=(kt == 0), stop=(kt == KT - 1),
                )
            if on_act:
                nc.scalar.activation(
                    qk[:, nt, mc * 512:(mc + 1) * 512], ps, AF.Identity,
                    bias=battn_t[:, nt:nt + 1], scale=1.0)
            else:
                nc.vector.tensor_scalar_add(
                    qk[:, nt, mc * 512:(mc + 1) * 512], ps,
                    battn_t[:, nt:nt + 1])

        def b2_mm(p, gi):
            nt, mc = (p, 6 + p)[gi // 2], gi % 2
            def go():
                ps = ps_mm.tile([128, 512], F32, tag="mm",
                                name=f"b2_{p}_{nt}_{mc}")
                emit_b2_group(nt, mc, ps, on_act=(gi % 2 == 1))
            return go

        def emit_d(mt):
            out_sb = io.tile([128, C], F32, tag="io", name=f"d_sb_{mt}")
            ps2 = [ps_mm.tile([128, 512], F32, tag="mm",
                              name=f"d_mm_{mt}_{g}") for g in range(2)]
            for gi, (c0, csz) in enumerate([(0, 512), (512, 256)]):
                for nt in range(KT):
                    nc.tensor.matmul(
                        ps2[gi][:, 0:csz],
                        yT[:, nt, mt * 128:(mt + 1) * 128],
                        wpr[:, nt, c0:c0 + csz],
                        start=(nt == 0), stop=(nt == KT - 1),
                    )
            for gi, (c0, csz) in enumerate([(0, 512), (512, 256)]):
                nc.vector.tensor_add(
                    out_sb[:, c0:c0 + csz], ps2[gi][:, 0:csz],
                    bp_b[:, c0:c0 + csz])
            nc.sync.dma_start(
                out=out[mt * 128:(mt + 1) * 128, :], in_=out_sb)

        # B1 fully upfront (PE-dense, warms HAM), then B2 pair 0
        for mt in range(MT):
            b1_st(mt)
        for nt in (0, 6):
            ps = ps_st.tile([128, 1024], F32, tag="st", name=f"b2p0_{nt}")
            for mc in range(2):
                emit_b2_group(nt, mc, ps[:, mc * 512:(mc + 1) * 512])

        # ---- phase C: attention, chunk-major per head pair ----
        def emit_qk(p, du_i, a, c):
            # head hh owns bank hh of st (cols hh*512 : hh*512+512) so the
            # two row-tiled concurrent matmuls never share a PSUM bank.
            st = ps_st.tile([128, 1024], F32, tag="st", name=f"st_{p}_{du_i}")
            isl = slice(c * ICH, (c + 1) * ICH)
            for half, jt in enumerate((2 * a, 2 * a + 1)):
                for hh, po in enumerate((0, 64)):
                    nc.tensor.matmul(
                        st[:, hh * 512 + half * 256:hh * 512 + (half + 1) * 256],
                        qk[po:po + 64, 6 + p, jt * 128:(jt + 1) * 128],
                        qk[po:po + 64, p, isl],
                        start=True, stop=True,
                    )
            ptile = pt_pool.tile([128, 1024], BF16, tag="pt",
                                 name=f"pt_{p}_{du_i}")
            nc.scalar.activation(ptile, st, AF.Exp, bias=0.0, scale=SCALE)
            if a == c:  # diagonal double-unit
                nc.vector.tensor_mul(ptile[:, 0:512], ptile[:, 0:512],
                                     maskC2[:, 0:512])
                nc.vector.tensor_mul(ptile[:, 512:1024], ptile[:, 512:1024],
                                     maskC2[:, 512:1024])
            return ptile

        def emit_pv(p, a, c, ptile, ya):
            # both heads accumulate into ONE ya bank: cols hh*ICH. Only the
            # first matmul of the chunk clears has_written (whole bank);
            # everything else accumulates / first-writes per element.
            for half, jt in enumerate((2 * a, 2 * a + 1)):
                for hh in range(2):
                    nc.tensor.matmul(
                        ya[:, hh * ICH:(hh + 1) * ICH],
                        v_aug[:, jt, 2 * p + hh, :],
                        ptile[:, hh * 512 + half * 256:hh * 512 + (half + 1) * 256],
                        start=(a == 0 and half == 0 and hh == 0),
                        stop=(a == c and half == 1 and hh == 1),
                        skip_group_check=True,
                    )

        def emit_tail(p, c, ya):
            isl = slice(c * ICH, (c + 1) * ICH)
            # custom-DVE recip is SBUF-only on HW: bounce l out of PSUM
            lrow = rl_pool.tile([1, 2 * ICH], F32, tag="lrow",
                                name=f"lrow_{p}_{c}")
            nc.vector.tensor_copy(lrow, ya[HS:HS + 1, :])
            rl = rl_pool.tile([1, 2 * ICH], F32, tag="rl", name=f"rl_{p}_{c}")
            nc.vector.reciprocal_approx_fast(out=rl, in_=lrow)
            rlb = rl_pool.tile([64, 2 * ICH], F32, tag="rlb",
                               name=f"rlb_{p}_{c}")
            nc.gpsimd.partition_broadcast(rlb, rl)
            for hh in range(2):
                po = 64 * hh
                nc.vector.tensor_mul(
                    yT[po:po + 64, p, isl],
                    ya[0:HS, hh * ICH:(hh + 1) * ICH],
                    rlb[:, hh * ICH:(hh + 1) * ICH])

        # Global software-pipelined du stream (lookahead 2).  One B2 group
        # for the next pair is emitted at each chunk close.
        stream = [(p, a, c) for p in range(6)
                  for c in range(NCH) for a in range(c + 1)]

        LOOKAHEAD = 3
        pending = []  # (p, a, c, ptile, yas)
        yas_by_chunk = {}
        b2_next = {p: iter([b2_mm(p + 1, gi) for gi in range(4)])
                   for p in range(5)}

        def pop_one():
            p, a, c, ptile, ya = pending.pop(0)
            emit_pv(p, a, c, ptile, ya)
            if a == c:
                emit_tail(p, c, ya)
                if p < 5 and c < 2:  # 2 B2 groups at each of first 2 closes
                    next(b2_next[p])()
                    next(b2_next[p])()

        for gi_, (p, a, c) in enumerate(stream):
            if a == 0:
                yas_by_chunk[(p, c)] = ps_ya.tile(
                    [HS + 1, 2 * ICH], F32, tag="ya", name=f"ya_{p}_{c}")
            ptile = emit_qk(p, gi_, a, c)
            pending.append((p, a, c, ptile, yas_by_chunk[(p, c)]))
            if len(pending) > LOOKAHEAD:
                pop_one()
        while pending:
            pop_one()

        # ---- phase D: output projection at the end, on st slots ----
        for mt in range(MT):
            out_sb = io.tile([128, C], F32, tag="io", name=f"d_sb_{mt}")
            ps = ps_st.tile([128, 1024], F32, tag="st", name=f"d_ps_{mt}")
            for gi, (c0, csz) in enumerate([(0, 512), (512, 256)]):
                for nt in range(KT):
                    nc.tensor.matmul(
                        ps[:, gi * 512:gi * 512 + csz],
                        yT[:, nt, mt * 128:(mt + 1) * 128],
                        wpr[:, nt, c0:c0 + csz],
                        start=(nt == 0), stop=(nt == KT - 1),
                    )
            for gi, (c0, csz) in enumerate([(0, 512), (512, 256)]):
                nc.vector.tensor_add(
                    out_sb[:, c0:c0 + csz], ps[:, gi * 512:gi * 512 + csz],
                    bp_b[:, c0:c0 + csz])
            nc.sync.dma_start(
                out=out[mt * 128:(mt + 1) * 128, :], in_=out_sb)

    nc.compile()
    return nc


_CACHE = {}


def _get_program():
    if "nc" not in _CACHE:
        _CACHE["nc"] = build_program()
    return _CACHE["nc"]


def kernel(x, w_attn, b_attn, w_proj, b_proj):
    nc = _get_program()
    x = np.asarray(x, dtype=np.float32)
    in_maps = [
        {
            "x": np.ascontiguousarray(x[b]),
            "w_attn": np.asarray(w_attn, np.float32),
            "b_attn": np.asarray(b_attn, np.float32),
            "w_proj": np.asarray(w_proj, np.float32),
            "b_proj": np.asarray(b_proj, np.float32),
        }
        for b in range(B)
    ]
    res = run_bass_kernel_spmd(nc, in_maps, list(range(N_CORES)))
    return np.stack([res.results[b]["out"] for b in range(B)], axis=0)
